# revision 25
# baseline (speedup 1.0000x reference)
"""Trainium2 Bass kernel for nn_LorentzLayer.

Math: the reference applies a per-cluster weighted Lorentz boost to T[b,c,:],
sums over clusters, then applies a second (inner) boost. Both boosts compose
into a single tiny matrix Mfull (400, 4) applied to T flattened to
(262144, 400):  out = Tf @ Mfull.

Device strategy (8 cores, pure batch data-parallel), v4 (fp8 DoubleRow +
persistent weights):
  - Host computes Mfull in float64 (it only depends on the tiny inputs).
  - T is streamed as fp8 e4m3 with error-feedback shaped rounding: each
    element is rounded to the e4m3 grid neighbor that minimizes the
    accumulated 4-vector output error sum_j M[j,:]*e[b,j]. 1 byte/elem
    (4x less HBM traffic than fp32-exact) at 3.3e-3 output rel-l2 error
    (gate 2e-2; plain RNE e4m3 would be 2.8e-2).
  - Mfull kept near-exact via an e4m3 hi plane + e4m3 lo plane pre-scaled
    by 16 (host divides the lo output rows back by 16).
  - Matmuls run in fp8 DoubleRow mode (2 fp8 weights/cell, 2 MACs/cycle).
    K=400 split: A=256 (128 part x 2, PE col strip 0), B=128 (64 part x 2,
    strip 1, SBUF base alternating 0/64 per subtile for even/odd SDMA
    balance), C=16 (8 part x 2, strip 2, base 96). Stationaries are padded
    to 32 columns (zeros) so psum partitions 0:96 are fully initialized and
    one fused PSUM->SBUF copy per block suffices.
  - The three strips hold their stationaries in DISJOINT PE array regions,
    so after the first load per pass the weights persist. bass lowers each
    matmul to InstLdweights+InstMatmult(ldweights=False); a post-compile
    pass drops redundant InstLdweights (same weights AP/tile_position),
    preserving their semaphore waits/updates as InstEventSemaphore. This
    removes the per-matmul weight-reload serialization (~470ns/mm -> ~110).
  - DRAM input layout is subtile-major: each subtile's bytes are one fully
    contiguous region per tensor, so every dma_start is a pure sequential
    HBM stream (128 descriptors x 8KB).
  - PSUM->SBUF copies convert to fp16 (output 1.5 MB/core) alternating
    DVE tensor_copy / ACT activation-Copy; input DMAs split across both
    HWDGE rings byte-balanced by subtile parity.
"""

import numpy as np
import ml_dtypes

E4 = ml_dtypes.float8_e4m3
F16 = np.float16

BATCH = 262144
CLUSTER = 100
KDIM = 4 * CLUSTER  # 400
NCORES = 8
B_CORE = BATCH // NCORES  # 32768
NB = 4096    # batch subtile (columns per DMA)
NPS = 512    # psum tile free size
LO_SCALE = 16.0  # stationary lo plane pre-scale (host divides back)
KA, KB, KC = 256, 128, 16   # DoubleRow K splits (KA+KB+KC == KDIM)
PA, PB, PC = KA // 2, KB // 2, KC // 2  # partitions per matmul
QC = 96      # rag base partition (strip 2 rows 96:104)


def _dedup_ldweights(nc):
    """Drop InstLdweights whose (weights AP, tile_position, perf_mode) was
    already loaded earlier in the same block; the array region still holds
    those weights (disjoint strips never clobber each other). Waits/updates
    of a dropped load are preserved as an InstEventSemaphore."""
    import concourse.mybir as mybir
    n_dropped = 0
    for blk in nc.main_func.blocks:
        seen = set()
        new_insts = []
        for inst in blk.instructions:
            if isinstance(inst, mybir.InstLdweights):
                key = (repr(inst.ins[0]), repr(inst.tile_position),
                       repr(inst.perf_mode))
                if key in seen:
                    n_dropped += 1
                    si = inst.sync_info
                    if si is not None and (len(si.on_wait) > 0
                                           or len(si.on_update) > 0):
                        ev = mybir.InstEventSemaphore(
                            name=nc.get_next_instruction_name(),
                            ins=[], outs=[])
                        ev.engine = inst.engine
                        ev.sync_info = mybir.SyncInfo(
                            on_wait=si.on_wait, on_update=si.on_update)
                        nc.register_instruction(ev)
                        new_insts.append(ev)
                    continue
                seen.add(key)
            new_insts.append(inst)
        blk.instructions[:] = new_insts
    return n_dropped


def _build_nc(b_core: int, nb: int, repeat: int = 1, mode: str = "full",
              bufs_in: int = 4, bufs_ps: int = 2, bufs_out: int = 4,
              copy_split: bool = True, split_rings: bool = True,
              kgrouped: bool = True, dedup_ldw: bool = True,
              do_copies: bool = True, half_split: bool = False,
              store_swdge: bool = True):
    """mode: 'full' | 'dma' (loads only) | 'compute' (no big loads).
    repeat>1 wraps the pass in a device-side For_i loop (timing harness)."""
    import concourse.bacc as bacc
    import concourse.tile as tile
    import concourse.mybir as mybir

    e4 = mybir.dt.float8e4
    f16 = mybir.dt.float16
    f32 = mybir.dt.float32
    Copy = mybir.ActivationFunctionType.Copy
    DR = mybir.MatmulPerfMode.DoubleRow

    nc = bacc.Bacc("TRN2", target_bir_lowering=False, debug=False, num_devices=NCORES)

    n_sub = b_core // nb
    n_ps = nb // NPS

    # Subtile-major DRAM layout: tensor rows [s*P, (s+1)*P) hold subtile s
    # -> fully sequential HBM reads. c01 rows are [half0 | half1] pairs
    # (DoubleRow); c2/rag are plain one-row-per-K layouts.
    c01 = nc.dram_tensor("c01", (n_sub * PA, 2 * nb), e4, kind="ExternalInput")
    c2 = nc.dram_tensor("c2", (n_sub * KB, nb), e4, kind="ExternalInput")
    rag = nc.dram_tensor("rag", (n_sub * KC, nb), e4, kind="ExternalInput")
    stat_a = nc.dram_tensor("stat_a", (128, 2, 32), e4, kind="ExternalInput")
    stat_b = nc.dram_tensor("stat_b", (128, 32), e4, kind="ExternalInput")
    stat_c = nc.dram_tensor("stat_c", (128, 32), e4, kind="ExternalInput")
    outT = nc.dram_tensor("outT", (24, b_core), f16, kind="ExternalOutput")

    do_dma = mode in ("full", "dma")
    do_compute = mode in ("full", "compute")

    with tile.TileContext(nc) as tc:
        with (
            tc.tile_pool(name="statp", bufs=1) as statpool,
            tc.tile_pool(name="inp", bufs=bufs_in) as inpool,
            tc.tile_pool(name="ragp", bufs=2) as ragpool,
            tc.tile_pool(name="outp", bufs=bufs_out) as outpool,
            tc.tile_pool(name="ps", bufs=bufs_ps, space="PSUM") as pspool,
        ):
            sa = statpool.tile([128, 2, 32], e4)
            sb = statpool.tile([128, 32], e4)
            sc = statpool.tile([128, 32], e4)
            nc.sync.dma_start(out=sa[:, :, :], in_=stat_a[:, :, :])
            nc.sync.dma_start(out=sb[:, :], in_=stat_b[:, :])
            nc.sync.dma_start(out=sc[:, :], in_=stat_c[:, :])

            if not do_dma:
                dummy_in = statpool.tile([128, 2, nb], e4)
                nc.gpsimd.memset(dummy_in[:, :, :], 0)
                dummy_p = statpool.tile([128, nb], e4)
                nc.gpsimd.memset(dummy_p[:, :], 0)

            def load_eng(s, k):
                """Byte-balanced HWDGE ring schedule. k: 0=c01 (256nb B),
                1=c2 (128nb), 2=rag (16nb), 3..5=out stores (16nb each).
                Even s: SP={c01}, ACT={c2,rag,outs}; odd s swaps."""
                if not split_rings:
                    return nc.sync
                if k == 0:
                    return nc.sync if s % 2 == 0 else nc.scalar
                return nc.scalar if s % 2 == 0 else nc.sync

            def pass_body():
                for s in range(n_sub):
                    ssl = slice(s * nb, (s + 1) * nb)
                    if do_dma:
                        ta = inpool.tile([128, 2, nb], e4, tag="c01")
                        tb = inpool.tile([128, nb], e4, tag="c2")
                        rt = ragpool.tile([128, nb], e4)
                        if half_split:
                            # partitions 0:64 -> even SDMA engines via SP,
                            # 64:128 -> odd via ACT: both rings balanced
                            # within every subtile.
                            nc.sync.dma_start(
                                out=ta[0:64, :, :],
                                in_=c01[s * PA:s * PA + 64, :])
                            nc.scalar.dma_start(
                                out=ta[64:128, :, :],
                                in_=c01[s * PA + 64:(s + 1) * PA, :])
                            nc.sync.dma_start(
                                out=tb[0:64, :],
                                in_=c2[s * KB:s * KB + 64, :])
                            nc.scalar.dma_start(
                                out=tb[64:128, :],
                                in_=c2[s * KB + 64:(s + 1) * KB, :])
                            load_eng(s, 2).dma_start(
                                out=rt[QC:QC + KC, :],
                                in_=rag[s * KC:(s + 1) * KC, :])
                        else:
                            load_eng(s, 0).dma_start(
                                out=ta[:, :, :],
                                in_=c01[s * PA:(s + 1) * PA, :])
                            load_eng(s, 1).dma_start(
                                out=tb[:, :],
                                in_=c2[s * KB:(s + 1) * KB, :])
                            load_eng(s, 2).dma_start(
                                out=rt[QC:QC + KC, :],
                                in_=rag[s * KC:(s + 1) * KC, :])
                    else:
                        ta = dummy_in
                        tb = rt = dummy_p
                    ot = outpool.tile([72, nb], f16)
                    if not do_compute:
                        nc.gpsimd.memset(ot[:, 0:1], 0)
                    if do_compute:
                        # Three strips (PE col groups 0/1/2 -> psum 0:32,
                        # 32:64, 64:96), each mm its own one-shot psum
                        # group; stationaries persist per strip (the
                        # post-compile pass drops repeated loads). a is
                        # DoubleRow (must target psum base 0); b/c are
                        # plain matmuls (col tiling is DR-incompatible).
                        g = min(n_ps, 4)
                        for h in range(n_ps // g):
                            js = range(h * g, (h + 1) * g)
                            pss = {j: pspool.tile([96, NPS], f32,
                                                  name=f"ps{j % g}")
                                   for j in js}
                            jsls = {j: slice(j * NPS, (j + 1) * NPS)
                                    for j in js}
                            mms = [
                                lambda j: nc.tensor.matmul(
                                    pss[j][0:32, :], sa[:, :, :],
                                    ta[:, :, jsls[j]],
                                    start=True, stop=True, perf_mode=DR,
                                    tile_position=(0, 0)),
                                lambda j: nc.tensor.matmul(
                                    pss[j][32:64, :], sb[:, :],
                                    tb[:, jsls[j]],
                                    start=True, stop=True,
                                    tile_position=(0, 32),
                                    skip_group_check=True),
                                lambda j: nc.tensor.matmul(
                                    pss[j][64:96, :], sc[QC:QC + KC, :],
                                    rt[QC:QC + KC, jsls[j]],
                                    start=True, stop=True,
                                    tile_position=(QC, 64),
                                    skip_group_check=True),
                            ]
                            if kgrouped:
                                for mm in mms:
                                    for j in js:
                                        mm(j)
                            else:
                                for j in js:
                                    for mm in mms:
                                        mm(j)
                            for j in js:
                                if not do_copies:
                                    continue
                                if copy_split and j % 2 == 1:
                                    nc.scalar.activation(ot[:, jsls[j]],
                                                         pss[j][0:72, :],
                                                         Copy)
                                else:
                                    nc.vector.tensor_copy(ot[:, jsls[j]],
                                                          pss[j][0:72, :])
                    if do_dma:
                        # Stores go out the SWDGE (gpsimd) ring so a store
                        # waiting on copies never blocks the next subtile's
                        # loads behind it in a FIFO HWDGE ring.
                        se = (lambda k: nc.gpsimd) if store_swdge else \
                            (lambda k: load_eng(s, k))
                        se(3).dma_start(out=outT[0:8, ssl],
                                        in_=ot[0:8, :])
                        se(4).dma_start(out=outT[8:16, ssl],
                                        in_=ot[32:40, :])
                        se(5).dma_start(out=outT[16:24, ssl],
                                        in_=ot[64:72, :])

            if repeat > 1:
                with tc.For_i(0, repeat, 1,
                              hint_engines=(mybir.EngineType.PE,
                                            mybir.EngineType.DVE,
                                            mybir.EngineType.SP,
                                            mybir.EngineType.Activation)):
                    pass_body()
            else:
                pass_body()

    nc.compile()
    if dedup_ldw:
        _dedup_ldweights(nc)
    return nc


def _boost_mats(boosts: np.ndarray, K_mats: np.ndarray) -> np.ndarray:
    """boosts (C,3) -> Lorentz boost matrices (C,4,4), float64."""
    b = boosts.astype(np.float64)
    K = K_mats.astype(np.float64)
    mag = np.sqrt((b * b).sum(axis=1, keepdims=True))        # (C,1)
    n = b / mag                                              # (C,3)
    g = 1.0 / np.sqrt(1.0 - mag * mag)                       # (C,1)
    nK = np.einsum('cj,jad->cad', n, K)                      # (C,4,4)
    nK2 = np.einsum('cab,cbd->cad', nK, nK)                  # (C,4,4)
    B = (np.eye(4)[None]
         - (g * mag)[..., None] * nK
         + (g - 1.0)[..., None] * nK2)
    return B


def _mfull(Bo, Bi, W, K_mats) -> np.ndarray:
    """Composite matrix Mfull (400, 4): out[b,a] = sum_j Tf[b,j] Mfull[j,a]."""
    Bc = _boost_mats(Bo, K_mats)                  # (C,4,4)
    B2 = _boost_mats(Bi, K_mats)[0]               # (4,4)
    comp = np.einsum('ad,cde->cae', B2, Bc)       # (C,4,4) = B2 @ Bc
    comp = comp * W.astype(np.float64)[:, None]   # weight per cluster
    # Mfull[c*4+d, a] = comp[c, a, d]
    return np.ascontiguousarray(comp.transpose(0, 2, 1).reshape(KDIM, 4))


def _m_planes(Mfull64: np.ndarray):
    """e4m3 hi/lo planes (each (KDIM,4)) and the effective float64 matrix."""
    M32 = Mfull64.astype(np.float32)
    Mhi = M32.astype(E4)
    Mlo = ((M32 - Mhi.astype(np.float32)) * LO_SCALE).astype(E4)
    Meff = Mhi.astype(np.float64) + Mlo.astype(np.float64) / LO_SCALE
    return Mhi, Mlo, Meff


def _pack_stat_dr(Mhi, Mlo, k0, P):
    """DoubleRow stationary (128, 2, 32) for K rows [k0, k0+2P): half i at
    [:, i, :], cols [4 hi | 4 lo | 24 zero]."""
    st = np.zeros((128, 2, 32), dtype=E4)
    for i in range(2):
        rows = slice(k0 + i * P, k0 + (i + 1) * P)
        st[0:P, i, 0:4] = Mhi[rows]
        st[0:P, i, 4:8] = Mlo[rows]
    return st


def _pack_stat_plain(Mhi, Mlo, k0, kn, base):
    """Plain stationary (128, 32) for K rows [k0, k0+kn) at partition
    `base`, cols [4 hi | 4 lo | 24 zero]."""
    st = np.zeros((128, 32), dtype=E4)
    st[base:base + kn, 0:4] = Mhi[k0:k0 + kn]
    st[base:base + kn, 4:8] = Mlo[k0:k0 + kn]
    return st


# e4m3 finite grid, ascending (for shaped rounding)
_E4_CODES = np.unique(
    np.arange(256, dtype=np.uint8).view(E4).astype(np.float64))
_E4_CODES = np.ascontiguousarray(
    _E4_CODES[np.isfinite(_E4_CODES)].astype(np.float32))


def _shape_chunk(args):
    """Error-feedback rounding of Tf chunk (n, 400) against Meff (400, 4).
    Greedy: pick the neighbor grid point minimizing ||r + M_j * e||^2."""
    Tc, Meff32 = args
    n = Tc.shape[0]
    codes = _E4_CODES
    r = np.zeros((n, 4), dtype=np.float32)
    out = np.empty((n, KDIM), dtype=E4)
    m2 = (Meff32 * Meff32).sum(axis=1)            # (400,)
    for j in range(KDIM):
        x = Tc[:, j]
        idx = np.searchsorted(codes, x).clip(1, len(codes) - 1)
        lo = codes[idx - 1]
        hi = codes[idx]
        e_lo = lo - x
        e_hi = hi - x
        Mj = Meff32[j]
        rm = r @ Mj
        take_hi = (2 * rm + m2[j] * (e_lo + e_hi)) * (e_hi - e_lo) < 0
        val = np.where(take_hi, hi, lo)
        r += (val - x)[:, None] * Mj[None, :]
        out[:, j] = val
    return out


def _shaped_quant(Tf: np.ndarray, Meff: np.ndarray) -> np.ndarray:
    """Shaped e4m3 quantization of Tf (BATCH, 400), parallel over batch."""
    Meff32 = Meff.astype(np.float32)
    nw = 16
    chunks = np.array_split(np.arange(BATCH), nw)
    args = [(Tf[c[0]:c[-1] + 1], Meff32) for c in chunks]
    try:
        import multiprocessing as mp
        with mp.get_context("fork").Pool(nw) as pool:
            parts = pool.map(_shape_chunk, args)
    except Exception:
        parts = [_shape_chunk(a) for a in args]
    return np.concatenate(parts, axis=0)


_NC_CACHE = {}


def _get_nc():
    key = (B_CORE, NB)
    if key not in _NC_CACHE:
        _NC_CACHE[key] = _build_nc(B_CORE, NB)
    return _NC_CACHE[key]


def _combine_out(o16: np.ndarray) -> np.ndarray:
    """(24, n) fp16 raw rows -> (n, 4) f32. Rows per strip: [hi(4), lo(4)]."""
    o = o16.astype(np.float32)
    inv = np.float32(1.0 / LO_SCALE)
    return (o[0:4] + o[4:8] * inv
            + o[8:12] + o[12:16] * inv
            + o[16:20] + o[20:24] * inv).T


def _plane_split(Tt: np.ndarray, nb: int):
    """(400, n) e4m3 -> subtile-major tensors: c01 pairs (n_sub*128, 2*nb),
    c2 plain (n_sub*128, nb), rag plain (n_sub*16, nb)."""
    n = Tt.shape[1]
    n_sub = n // nb

    def pack_dr(rows, P):
        return np.ascontiguousarray(
            rows.reshape(2, P, n_sub, nb).transpose(2, 1, 0, 3)
            .reshape(n_sub * P, 2 * nb))

    def pack_plain(rows, kn):
        return np.ascontiguousarray(
            rows.reshape(kn, n_sub, nb).transpose(1, 0, 2)
            .reshape(n_sub * kn, nb))

    return (pack_dr(Tt[0:KA], PA), pack_plain(Tt[KA:KA + KB], KB),
            pack_plain(Tt[KA + KB:], KC))


def _selftest_small():
    """CoreSim structural/numeric check at reduced size (no hardware)."""
    from concourse.bass_interp import CoreSim
    b_core_t, nb_t = 2048, 512
    rng = np.random.default_rng(0)
    Tt = rng.standard_normal((KDIM, b_core_t)).astype(np.float32)
    Mfull = rng.standard_normal((KDIM, 4)).astype(np.float64) * 0.3
    Mhi, Mlo, Meff = _m_planes(Mfull)
    T8 = Tt.astype(E4)
    c01, c2, rg = _plane_split(T8, nb_t)
    nc = _build_nc(b_core_t, nb_t)
    sim = CoreSim(nc, require_finite=True, require_nnan=True)
    sim.tensor("stat_a")[:] = _pack_stat_dr(Mhi, Mlo, 0, PA)
    sim.tensor("stat_b")[:] = _pack_stat_plain(Mhi, Mlo, KA, KB, 0)
    sim.tensor("stat_c")[:] = _pack_stat_plain(Mhi, Mlo, KA + KB, KC, QC)
    sim.tensor("c01")[:] = c01
    sim.tensor("c2")[:] = c2
    sim.tensor("rag")[:] = rg
    sim.simulate(check_with_hw=False)
    got = _combine_out(np.asarray(sim.tensor("outT")))
    want = T8.astype(np.float64).T @ Meff
    rel = np.linalg.norm(got - want) / np.linalg.norm(want)
    assert rel < 2e-3, rel
    return rel


def prepare_in_maps(T, Bo, Bi, W, K_mats):
    T = np.asarray(T, dtype=np.float32)
    Mfull = _mfull(np.asarray(Bo), np.asarray(Bi),
                   np.asarray(W), np.asarray(K_mats))
    Mhi, Mlo, Meff = _m_planes(Mfull)
    sa = _pack_stat_dr(Mhi, Mlo, 0, PA)
    sb = _pack_stat_plain(Mhi, Mlo, KA, KB, 0)
    sc = _pack_stat_plain(Mhi, Mlo, KA + KB, KC, QC)
    Tq = _shaped_quant(T.reshape(BATCH, KDIM), Meff)
    in_maps = []
    for c in range(NCORES):
        Tt = np.ascontiguousarray(Tq[c * B_CORE:(c + 1) * B_CORE].T)
        c01, c2, rg = _plane_split(Tt, NB)
        in_maps.append({"c01": c01, "c2": c2, "rag": rg,
                        "stat_a": sa, "stat_b": sb, "stat_c": sc})
    return in_maps


# Set by test harnesses to profile the run; kernel() stores the spmd results
# object (exec_time_ns etc.) in LAST_RESULTS when TRACE is on.
TRACE = False
TRACE_KWARGS = {}
LAST_RESULTS = None


def kernel(T, Bo, Bi, W, K_mats):
    from concourse.bass_utils import run_bass_kernel_spmd

    in_maps = prepare_in_maps(T, Bo, Bi, W, K_mats)
    nc = _get_nc()
    res = run_bass_kernel_spmd(nc, in_maps, core_ids=list(range(NCORES)),
                               trace=TRACE, **TRACE_KWARGS)
    if TRACE:
        global LAST_RESULTS
        LAST_RESULTS = res

    out = np.empty((BATCH, 4), dtype=np.float32)
    for c in range(NCORES):
        out[c * B_CORE:(c + 1) * B_CORE] = _combine_out(res.results[c]["outT"])
    return out.reshape(BATCH, 1, 4)


if __name__ == "__main__":
    print("selftest rel:", _selftest_small())


# revision 33
# speedup vs baseline: 1.2024x; 1.2024x over previous
"""Trainium2 Bass kernel for nn_LorentzLayer.

Math: the reference applies a per-cluster weighted Lorentz boost to T[b,c,:],
sums over clusters, then applies a second (inner) boost. Both boosts compose
into a single tiny matrix Mfull (400, 4) applied to T flattened to
(262144, 400):  out = Tf @ Mfull.

Device strategy (8 cores, pure batch data-parallel), v4 (fp8 DoubleRow +
persistent weights):
  - Host computes Mfull in float64 (it only depends on the tiny inputs).
  - T is streamed as fp8 e4m3 with error-feedback shaped rounding: each
    element is rounded to the e4m3 grid neighbor that minimizes the
    accumulated 4-vector output error sum_j M[j,:]*e[b,j]. 1 byte/elem
    (4x less HBM traffic than fp32-exact) at 3.3e-3 output rel-l2 error
    (gate 2e-2; plain RNE e4m3 would be 2.8e-2).
  - Mfull kept near-exact via an e4m3 hi plane + e4m3 lo plane pre-scaled
    by 16 (host divides the lo output rows back by 16).
  - Matmuls run in fp8 DoubleRow mode (2 fp8 weights/cell, 2 MACs/cycle).
    K=400 split: A=256 (128 part x 2, PE col strip 0), B=128 (64 part x 2,
    strip 1, SBUF base alternating 0/64 per subtile for even/odd SDMA
    balance), C=16 (8 part x 2, strip 2, base 96). Stationaries are padded
    to 32 columns (zeros) so psum partitions 0:96 are fully initialized and
    one fused PSUM->SBUF copy per block suffices.
  - The three strips hold their stationaries in DISJOINT PE array regions,
    so after the first load per pass the weights persist. bass lowers each
    matmul to InstLdweights+InstMatmult(ldweights=False); a post-compile
    pass drops redundant InstLdweights (same weights AP/tile_position),
    preserving their semaphore waits/updates as InstEventSemaphore. This
    removes the per-matmul weight-reload serialization (~470ns/mm -> ~110).
  - DRAM input layout is subtile-major: each subtile's bytes are one fully
    contiguous region per tensor, so every dma_start is a pure sequential
    HBM stream (128 descriptors x 8KB).
  - PSUM->SBUF copies convert to fp16 (output 1.5 MB/core) alternating
    DVE tensor_copy / ACT activation-Copy; input DMAs split across both
    HWDGE rings byte-balanced by subtile parity.
"""

import numpy as np
import ml_dtypes

E4 = ml_dtypes.float8_e4m3
F16 = np.float16

BATCH = 262144
CLUSTER = 100
KDIM = 4 * CLUSTER  # 400
NCORES = 8
B_CORE = BATCH // NCORES  # 32768
NB = 4096    # batch subtile (columns per DMA)
NPS = 512    # psum tile free size
LO_SCALE = 16.0  # stationary lo plane pre-scale (host divides back)
KA, KB, KC = 256, 128, 16   # DoubleRow K splits (KA+KB+KC == KDIM)
PA, PB, PC = KA // 2, KB // 2, KC // 2  # partitions per matmul
QC = 96      # rag base partition (strip 2 rows 96:104)


def _dedup_ldweights(nc):
    """Drop InstLdweights whose (weights AP, tile_position, perf_mode) was
    already loaded earlier in the same block; the array region still holds
    those weights (disjoint strips never clobber each other). Waits/updates
    of a dropped load are preserved as an InstEventSemaphore."""
    import concourse.mybir as mybir
    n_dropped = 0
    for blk in nc.main_func.blocks:
        seen = set()
        new_insts = []
        for inst in blk.instructions:
            if isinstance(inst, mybir.InstLdweights):
                key = (repr(inst.ins[0]), repr(inst.tile_position),
                       repr(inst.perf_mode))
                if key in seen:
                    n_dropped += 1
                    si = inst.sync_info
                    if si is not None and (len(si.on_wait) > 0
                                           or len(si.on_update) > 0):
                        ev = mybir.InstEventSemaphore(
                            name=nc.get_next_instruction_name(),
                            ins=[], outs=[])
                        ev.engine = inst.engine
                        ev.sync_info = mybir.SyncInfo(
                            on_wait=si.on_wait, on_update=si.on_update)
                        nc.register_instruction(ev)
                        new_insts.append(ev)
                    continue
                seen.add(key)
            new_insts.append(inst)
        blk.instructions[:] = new_insts
    return n_dropped


def _build_nc(b_core: int, nb: int, repeat: int = 1, mode: str = "full",
              bufs_in: int = 6, bufs_ps: int = 2, bufs_out: int = 4,
              copy_split: bool = True, split_rings: bool = True,
              kgrouped: bool = True, dedup_ldw: bool = True,
              do_copies: bool = True, half_split: bool = False,
              store_swdge: bool = True, unroll: int | None = None):
    """mode: 'full' | 'dma' (loads only) | 'compute' (no big loads).
    repeat>1 wraps the pass in a device-side For_i loop (timing harness)."""
    import concourse.bacc as bacc
    import concourse.tile as tile
    import concourse.mybir as mybir

    e4 = mybir.dt.float8e4
    f16 = mybir.dt.float16
    f32 = mybir.dt.float32
    Copy = mybir.ActivationFunctionType.Copy
    DR = mybir.MatmulPerfMode.DoubleRow

    nc = bacc.Bacc("TRN2", target_bir_lowering=False, debug=False, num_devices=NCORES)

    n_sub = b_core // nb
    n_ps = nb // NPS

    # Subtile-major DRAM layout: tensor rows [s*P, (s+1)*P) hold subtile s
    # -> fully sequential HBM reads. c01 rows are [half0 | half1] pairs
    # (DoubleRow); c2/rag are plain one-row-per-K layouts.
    c01 = nc.dram_tensor("c01", (n_sub * PA, 2 * nb), e4, kind="ExternalInput")
    c2 = nc.dram_tensor("c2", (n_sub * KB, nb), e4, kind="ExternalInput")
    rag = nc.dram_tensor("rag", (n_sub * KC, nb), e4, kind="ExternalInput")
    stat_a = nc.dram_tensor("stat_a", (128, 2, 32), e4, kind="ExternalInput")
    stat_b = nc.dram_tensor("stat_b", (128, 32), e4, kind="ExternalInput")
    stat_c = nc.dram_tensor("stat_c", (128, 32), e4, kind="ExternalInput")
    outT = nc.dram_tensor("outT", (24, b_core), f16, kind="ExternalOutput")

    do_dma = mode in ("full", "dma")
    do_compute = mode in ("full", "compute")

    with tile.TileContext(nc) as tc:
        with (
            tc.tile_pool(name="statp", bufs=1) as statpool,
            tc.tile_pool(name="inp", bufs=bufs_in) as inpool,
            tc.tile_pool(name="ragp", bufs=bufs_in) as ragpool,
            tc.tile_pool(name="outp", bufs=bufs_out) as outpool,
            tc.tile_pool(name="ps", bufs=bufs_ps, space="PSUM") as pspool,
        ):
            sa = statpool.tile([128, 2, 32], e4)
            sb = statpool.tile([128, 32], e4)
            sc = statpool.tile([128, 32], e4)
            nc.sync.dma_start(out=sa[:, :, :], in_=stat_a[:, :, :])
            nc.sync.dma_start(out=sb[:, :], in_=stat_b[:, :])
            nc.sync.dma_start(out=sc[:, :], in_=stat_c[:, :])

            if not do_dma:
                dummy_in = statpool.tile([128, 2, nb], e4)
                nc.gpsimd.memset(dummy_in[:, :, :], 0)
                dummy_p = statpool.tile([128, nb], e4)
                nc.gpsimd.memset(dummy_p[:, :], 0)

            def load_eng(s, k):
                """Byte-balanced HWDGE ring schedule. k: 0=c01 (256nb B),
                1=c2 (128nb), 2=rag (16nb), 3..5=out stores (16nb each).
                Even s: SP={c01}, ACT={c2,rag,outs}; odd s swaps."""
                if not split_rings:
                    return nc.sync
                if k == 0:
                    return nc.sync if s % 2 == 0 else nc.scalar
                return nc.scalar if s % 2 == 0 else nc.sync

            def issue_loads(s):
                ta = inpool.tile([128, 2, nb], e4, tag="c01")
                tb = inpool.tile([128, nb], e4, tag="c2")
                rt = ragpool.tile([128, nb], e4)
                load_eng(s, 0).dma_start(
                    out=ta[:, :, :],
                    in_=c01[s * PA:(s + 1) * PA, :])
                load_eng(s, 1).dma_start(
                    out=tb[:, :],
                    in_=c2[s * KB:(s + 1) * KB, :])
                load_eng(s, 2).dma_start(
                    out=rt[QC:QC + KC, :],
                    in_=rag[s * KC:(s + 1) * KC, :])
                return ta, tb, rt

            def compute_subtile(s, tiles):
                    ssl = slice(s * nb, (s + 1) * nb)
                    if do_dma:
                        ta, tb, rt = tiles
                    else:
                        ta = dummy_in
                        tb = rt = dummy_p
                    ot = outpool.tile([72, nb], f16)
                    if not do_compute:
                        nc.gpsimd.memset(ot[:, 0:1], 0)
                    if do_compute:
                        # Three strips (PE col groups 0/1/2 -> psum 0:32,
                        # 32:64, 64:96), each mm its own one-shot psum
                        # group; stationaries persist per strip (the
                        # post-compile pass drops repeated loads). a is
                        # DoubleRow (must target psum base 0); b/c are
                        # plain matmuls (col tiling is DR-incompatible).
                        g = min(n_ps, 4)
                        for h in range(n_ps // g):
                            js = range(h * g, (h + 1) * g)
                            pss = {j: pspool.tile([96, NPS], f32,
                                                  name=f"ps{j % g}")
                                   for j in js}
                            jsls = {j: slice(j * NPS, (j + 1) * NPS)
                                    for j in js}
                            mms = [
                                lambda j: nc.tensor.matmul(
                                    pss[j][0:32, :], sa[:, :, :],
                                    ta[:, :, jsls[j]],
                                    start=True, stop=True, perf_mode=DR,
                                    tile_position=(0, 0)),
                                lambda j: nc.tensor.matmul(
                                    pss[j][32:64, :], sb[:, :],
                                    tb[:, jsls[j]],
                                    start=True, stop=True,
                                    tile_position=(0, 32),
                                    skip_group_check=True),
                                lambda j: nc.tensor.matmul(
                                    pss[j][64:96, :], sc[QC:QC + KC, :],
                                    rt[QC:QC + KC, jsls[j]],
                                    start=True, stop=True,
                                    tile_position=(QC, 64),
                                    skip_group_check=True),
                            ]
                            if kgrouped:
                                for mm in mms:
                                    for j in js:
                                        mm(j)
                            else:
                                for j in js:
                                    for mm in mms:
                                        mm(j)
                            for j in js:
                                if not do_copies:
                                    continue
                                if copy_split and j % 2 == 1:
                                    nc.scalar.activation(ot[:, jsls[j]],
                                                         pss[j][0:72, :],
                                                         Copy)
                                else:
                                    nc.vector.tensor_copy(ot[:, jsls[j]],
                                                          pss[j][0:72, :])
                    if do_dma:
                        # Stores go out the SWDGE (gpsimd) ring so a store
                        # waiting on copies never blocks the next subtile's
                        # loads behind it in a FIFO HWDGE ring.
                        se = (lambda k: nc.gpsimd) if store_swdge else \
                            (lambda k: load_eng(s, k))
                        se(3).dma_start(out=outT[0:8, ssl],
                                        in_=ot[0:8, :])
                        se(4).dma_start(out=outT[8:16, ssl],
                                        in_=ot[32:40, :])
                        se(5).dma_start(out=outT[16:24, ssl],
                                        in_=ot[64:72, :])

            def pass_body(n_passes=1):
                # Software pipeline: loads issue PF subtiles ahead of
                # compute, so dma_starts sit ahead of the copies in the
                # FIFO engine queues (ACT queue depth is only 8).
                from collections import deque
                seq = [s for _ in range(n_passes) for s in range(n_sub)]
                PF = 2 if do_dma else 0
                pending = deque()
                for i, s in enumerate(seq):
                    if do_dma:
                        pending.append(issue_loads(s))
                    if i >= PF:
                        compute_subtile(seq[i - PF],
                                        pending.popleft() if do_dma else None)
                for k in range(PF):
                    i = len(seq) - PF + k
                    if i >= 0:
                        compute_subtile(seq[i], pending.popleft())

            if repeat > 1:
                # Unroll passes inside the hardware loop: consecutive
                # passes pipeline through the tile pools and the per-
                # iteration engine drains are amortized.
                if unroll is None:
                    unroll = 4 if repeat % 4 == 0 else \
                        (2 if repeat % 2 == 0 else 1)
                assert repeat % unroll == 0
                with tc.For_i(0, repeat // unroll, 1,
                              hint_engines=(mybir.EngineType.PE,
                                            mybir.EngineType.DVE,
                                            mybir.EngineType.SP,
                                            mybir.EngineType.Activation)):
                    pass_body(unroll)
            else:
                pass_body()

    nc.compile()
    if dedup_ldw:
        _dedup_ldweights(nc)
    return nc


def _boost_mats(boosts: np.ndarray, K_mats: np.ndarray) -> np.ndarray:
    """boosts (C,3) -> Lorentz boost matrices (C,4,4), float64."""
    b = boosts.astype(np.float64)
    K = K_mats.astype(np.float64)
    mag = np.sqrt((b * b).sum(axis=1, keepdims=True))        # (C,1)
    n = b / mag                                              # (C,3)
    g = 1.0 / np.sqrt(1.0 - mag * mag)                       # (C,1)
    nK = np.einsum('cj,jad->cad', n, K)                      # (C,4,4)
    nK2 = np.einsum('cab,cbd->cad', nK, nK)                  # (C,4,4)
    B = (np.eye(4)[None]
         - (g * mag)[..., None] * nK
         + (g - 1.0)[..., None] * nK2)
    return B


def _mfull(Bo, Bi, W, K_mats) -> np.ndarray:
    """Composite matrix Mfull (400, 4): out[b,a] = sum_j Tf[b,j] Mfull[j,a]."""
    Bc = _boost_mats(Bo, K_mats)                  # (C,4,4)
    B2 = _boost_mats(Bi, K_mats)[0]               # (4,4)
    comp = np.einsum('ad,cde->cae', B2, Bc)       # (C,4,4) = B2 @ Bc
    comp = comp * W.astype(np.float64)[:, None]   # weight per cluster
    # Mfull[c*4+d, a] = comp[c, a, d]
    return np.ascontiguousarray(comp.transpose(0, 2, 1).reshape(KDIM, 4))


def _m_planes(Mfull64: np.ndarray):
    """e4m3 hi/lo planes (each (KDIM,4)) and the effective float64 matrix."""
    M32 = Mfull64.astype(np.float32)
    Mhi = M32.astype(E4)
    Mlo = ((M32 - Mhi.astype(np.float32)) * LO_SCALE).astype(E4)
    Meff = Mhi.astype(np.float64) + Mlo.astype(np.float64) / LO_SCALE
    return Mhi, Mlo, Meff


def _pack_stat_dr(Mhi, Mlo, k0, P):
    """DoubleRow stationary (128, 2, 32) for K rows [k0, k0+2P): half i at
    [:, i, :], cols [4 hi | 4 lo | 24 zero]."""
    st = np.zeros((128, 2, 32), dtype=E4)
    for i in range(2):
        rows = slice(k0 + i * P, k0 + (i + 1) * P)
        st[0:P, i, 0:4] = Mhi[rows]
        st[0:P, i, 4:8] = Mlo[rows]
    return st


def _pack_stat_plain(Mhi, Mlo, k0, kn, base):
    """Plain stationary (128, 32) for K rows [k0, k0+kn) at partition
    `base`, cols [4 hi | 4 lo | 24 zero]."""
    st = np.zeros((128, 32), dtype=E4)
    st[base:base + kn, 0:4] = Mhi[k0:k0 + kn]
    st[base:base + kn, 4:8] = Mlo[k0:k0 + kn]
    return st


# e4m3 finite grid, ascending (for shaped rounding)
_E4_CODES = np.unique(
    np.arange(256, dtype=np.uint8).view(E4).astype(np.float64))
_E4_CODES = np.ascontiguousarray(
    _E4_CODES[np.isfinite(_E4_CODES)].astype(np.float32))


def _shape_chunk(args):
    """Error-feedback rounding of Tf chunk (n, 400) against Meff (400, 4).
    Greedy: pick the neighbor grid point minimizing ||r + M_j * e||^2."""
    Tc, Meff32 = args
    n = Tc.shape[0]
    codes = _E4_CODES
    r = np.zeros((n, 4), dtype=np.float32)
    out = np.empty((n, KDIM), dtype=E4)
    m2 = (Meff32 * Meff32).sum(axis=1)            # (400,)
    for j in range(KDIM):
        x = Tc[:, j]
        idx = np.searchsorted(codes, x).clip(1, len(codes) - 1)
        lo = codes[idx - 1]
        hi = codes[idx]
        e_lo = lo - x
        e_hi = hi - x
        Mj = Meff32[j]
        rm = r @ Mj
        take_hi = (2 * rm + m2[j] * (e_lo + e_hi)) * (e_hi - e_lo) < 0
        val = np.where(take_hi, hi, lo)
        r += (val - x)[:, None] * Mj[None, :]
        out[:, j] = val
    return out


def _shaped_quant(Tf: np.ndarray, Meff: np.ndarray) -> np.ndarray:
    """Shaped e4m3 quantization of Tf (BATCH, 400), parallel over batch
    when CPUs are available (fork+IPC is pure overhead on 1-2 cores)."""
    import os
    Meff32 = Meff.astype(np.float32)
    ncpu = len(os.sched_getaffinity(0)) if hasattr(os, "sched_getaffinity") \
        else (os.cpu_count() or 1)
    if ncpu <= 2:
        return _shape_chunk((Tf, Meff32))
    nw = min(16, ncpu)
    chunks = np.array_split(np.arange(BATCH), nw)
    args = [(Tf[c[0]:c[-1] + 1], Meff32) for c in chunks]
    try:
        import multiprocessing as mp
        with mp.get_context("fork").Pool(nw) as pool:
            parts = pool.map(_shape_chunk, args)
    except Exception:
        parts = [_shape_chunk(a) for a in args]
    return np.concatenate(parts, axis=0)


_NC_CACHE = {}


def _get_nc():
    key = (B_CORE, NB)
    if key not in _NC_CACHE:
        _NC_CACHE[key] = _build_nc(B_CORE, NB)
    return _NC_CACHE[key]


def _combine_out(o16: np.ndarray) -> np.ndarray:
    """(24, n) fp16 raw rows -> (n, 4) f32. Rows per strip: [hi(4), lo(4)]."""
    o = o16.astype(np.float32)
    inv = np.float32(1.0 / LO_SCALE)
    return (o[0:4] + o[4:8] * inv
            + o[8:12] + o[12:16] * inv
            + o[16:20] + o[20:24] * inv).T


def _plane_split(Tt: np.ndarray, nb: int):
    """(400, n) e4m3 -> subtile-major tensors: c01 pairs (n_sub*128, 2*nb),
    c2 plain (n_sub*128, nb), rag plain (n_sub*16, nb)."""
    n = Tt.shape[1]
    n_sub = n // nb

    def pack_dr(rows, P):
        return np.ascontiguousarray(
            rows.reshape(2, P, n_sub, nb).transpose(2, 1, 0, 3)
            .reshape(n_sub * P, 2 * nb))

    def pack_plain(rows, kn):
        return np.ascontiguousarray(
            rows.reshape(kn, n_sub, nb).transpose(1, 0, 2)
            .reshape(n_sub * kn, nb))

    return (pack_dr(Tt[0:KA], PA), pack_plain(Tt[KA:KA + KB], KB),
            pack_plain(Tt[KA + KB:], KC))


def _selftest_small():
    """CoreSim structural/numeric check at reduced size (no hardware)."""
    from concourse.bass_interp import CoreSim
    b_core_t, nb_t = 2048, 512
    rng = np.random.default_rng(0)
    Tt = rng.standard_normal((KDIM, b_core_t)).astype(np.float32)
    Mfull = rng.standard_normal((KDIM, 4)).astype(np.float64) * 0.3
    Mhi, Mlo, Meff = _m_planes(Mfull)
    T8 = Tt.astype(E4)
    c01, c2, rg = _plane_split(T8, nb_t)
    nc = _build_nc(b_core_t, nb_t)
    sim = CoreSim(nc, require_finite=True, require_nnan=True)
    sim.tensor("stat_a")[:] = _pack_stat_dr(Mhi, Mlo, 0, PA)
    sim.tensor("stat_b")[:] = _pack_stat_plain(Mhi, Mlo, KA, KB, 0)
    sim.tensor("stat_c")[:] = _pack_stat_plain(Mhi, Mlo, KA + KB, KC, QC)
    sim.tensor("c01")[:] = c01
    sim.tensor("c2")[:] = c2
    sim.tensor("rag")[:] = rg
    sim.simulate(check_with_hw=False)
    got = _combine_out(np.asarray(sim.tensor("outT")))
    want = T8.astype(np.float64).T @ Meff
    rel = np.linalg.norm(got - want) / np.linalg.norm(want)
    assert rel < 2e-3, rel
    return rel


def prepare_in_maps(T, Bo, Bi, W, K_mats):
    T = np.asarray(T, dtype=np.float32)
    Mfull = _mfull(np.asarray(Bo), np.asarray(Bi),
                   np.asarray(W), np.asarray(K_mats))
    Mhi, Mlo, Meff = _m_planes(Mfull)
    sa = _pack_stat_dr(Mhi, Mlo, 0, PA)
    sb = _pack_stat_plain(Mhi, Mlo, KA, KB, 0)
    sc = _pack_stat_plain(Mhi, Mlo, KA + KB, KC, QC)
    Tq = _shaped_quant(T.reshape(BATCH, KDIM), Meff)
    in_maps = []
    for c in range(NCORES):
        Tt = np.ascontiguousarray(Tq[c * B_CORE:(c + 1) * B_CORE].T)
        c01, c2, rg = _plane_split(Tt, NB)
        in_maps.append({"c01": c01, "c2": c2, "rag": rg,
                        "stat_a": sa, "stat_b": sb, "stat_c": sc})
    return in_maps


# Set by test harnesses to profile the run; kernel() stores the spmd results
# object (exec_time_ns etc.) in LAST_RESULTS when TRACE is on.
TRACE = False
TRACE_KWARGS = {}
LAST_RESULTS = None


def kernel(T, Bo, Bi, W, K_mats):
    from concourse.bass_utils import run_bass_kernel_spmd

    in_maps = prepare_in_maps(T, Bo, Bi, W, K_mats)
    nc = _get_nc()
    res = run_bass_kernel_spmd(nc, in_maps, core_ids=list(range(NCORES)),
                               trace=TRACE, **TRACE_KWARGS)
    if TRACE:
        global LAST_RESULTS
        LAST_RESULTS = res

    out = np.empty((BATCH, 4), dtype=np.float32)
    for c in range(NCORES):
        out[c * B_CORE:(c + 1) * B_CORE] = _combine_out(res.results[c]["outT"])
    return out.reshape(BATCH, 1, 4)


if __name__ == "__main__":
    print("selftest rel:", _selftest_small())
